# revision 20
# baseline (speedup 1.0000x reference)
"""ProxyNCA loss on 8 Trainium2 NeuronCores.

Math: with p_hat = p / ||p||, the reference
    loss_i = D2[i,t_i] + log sum_{k != t_i} exp(-D2[i,k])
with D2 = |x|^2 + |p_hat|^2 - 2 x.p_hat collapses (|x|^2 and |p_hat|^2 = 1
cancel between the two terms) to
    loss_i = -G[i,t_i] + log sum_{k != t_i} exp(G[i,k]),   G = 2 X Pn^T.

Device sharding: proxies split over classes across 8 cores (12500/core),
further split across SBUF partition halves (6250 columns on partitions
0-63 holding classes [0,6250), 6250 on partitions 64-127 holding classes
[6250,12500)) so nothing is duplicated. Per core:
  - norms^2 via a block-diagonal ones matmul (one 128-contraction matmul
    per 512 columns gives per-half norms in each partition half),
  - 1/||p|| via ACT Ln+Exp (exp(-0.5 ln n2 + ln 2) = 2/||p||) -- stays in
    the natural_log_exp table set with the main Exp, so no ACT table
    reloads inside the loop,
  - G computed as row-packed fp32r matmul pairs (tile_position (0,0) and
    (64,0)) into [128,2048] PSUM groups,
  - sum_k exp(G) split between two engines: ACT does groups 0-3 + the
    212-col remainder with fused exp+accumulate; the DVE does groups 4-5
    with a custom 2-pass op: pass1 w = (1 + u(c1 + u c2))^16 ~= e^(G/16)
    (deg-2 poly in u = G/256 + 4 squarings, 8 ALU stages), pass2
    w^16 + fused row-sum (4 squarings + accum stage).
The positive term G[i, t_i] is computed per batch shard from host-gathered
proxy rows. Host combines in float64: subtracts exp(pos) from the global
sum (exact masking) and averages.
"""

import numpy as np
from operator import add

import concourse.bacc as bacc
import concourse.mybir as mybir
import concourse.tile as tile
from concourse.bass_utils import run_bass_kernel_spmd

import concourse.dve_ops as dve_ops
import concourse.bacc as _bacc_mod
from concourse.hw_specs import get_activation_tables as _get_act_tables


def _act_tables_lnexp_first(arch):
    """Steer the ACT table-load pass to the combined natural_log_exp set so
    Ln+Exp kernels load one table instead of thrashing between natural_log
    and exp_and_others.  Order (and thus act_func_set_id numbering) is
    preserved -- ln/exp are only hidden from the non-combined sets so the
    chooser can't pick them."""
    import concourse.mybir as _mb
    t = _get_act_tables(arch)
    ln_f = _mb.ActivationFunctionType.Ln
    exp_f = _mb.ActivationFunctionType.Exp
    out = {}
    for k, fns in t.items():
        if "natural_log_exp" in k:
            out[k] = fns
        else:
            out[k] = {f for f in fns if f not in (ln_f, exp_f)}
    return out


_bacc_mod.get_activation_tables = _act_tables_lnexp_first
from concourse.dve_spec import Spec, Src0, C0, C1, Zero, One, lower, _has_src1, sq
from concourse.dve_uop import DveOpSpec

F32 = mybir.dt.float32
F32R = mybir.dt.float32r
AX = mybir.AxisListType.X
MULT = mybir.AluOpType.mult
EXP = mybir.ActivationFunctionType.Exp
LN = mybir.ActivationFunctionType.Ln

B, C, D = 1024, 100000, 64
NCORES = 8
CS = C // NCORES          # 12500 classes per core
CS2 = CS // 2             # 6250 classes per partition half
BS = B // NCORES          # 128 batch rows per core (positive extraction)
NBLK = B // 128           # 8 batch blocks of 128 rows

# exp(g) ~= ((1 + u(C1 + u C0))^16)^16 with u = g/256 (Taylor in u).
SCALE = 256.0
C1V = 1.0 / SCALE
C0V = 1.0 / (2.0 * SCALE * SCALE)

_CACHE = {}


# ---- custom DVE ops ------------------------------------------------------- #

def _ref_p1(in0, in1, c0, c1, c2):
    x = in0.astype(np.float32)
    c0 = np.float32(c0) if not isinstance(c0, np.ndarray) else c0.astype(np.float32)
    c1 = np.float32(c1) if not isinstance(c1, np.ndarray) else c1.astype(np.float32)
    w = ((x * c0 + c1) * x + np.float32(1.0)).astype(np.float32)
    for _ in range(4):
        w = (w * w).astype(np.float32)
    return w


def _ref_p2(in0, in1, c0, c1, c2):
    w = in0.astype(np.float32)
    for _ in range(4):
        w = (w * w).astype(np.float32)
    s = w.reshape(w.shape[0], -1).sum(axis=-1, keepdims=True)
    return w, s


def _register(name, spec, subdim=False):
    if name in dve_ops._SUB_OPCODE_FOR_NAME:
        for op in dve_ops.OPS:
            if op.name == name:
                return op
        raise RuntimeError(f"{name} registered but not in OPS")
    row = dve_ops._CUSTOM_DVE_ROW_BASE + len(dve_ops.OPS)
    assert row < 0x20
    dve_ops._SUB_OPCODE_FOR_NAME[name] = row
    shas = {}
    for ver in ("v3", "v4"):
        uops = lower(spec, ver=ver)
        shas[ver] = DveOpSpec(
            name=name, opcode=row, uops=uops, rd1_en=_has_src1(spec)
        ).sha(ver)
    op = dve_ops.DveOp(name, spec, subdim=subdim, uops_sha=shas)
    dve_ops.OPS.append(op)
    dve_ops.CUSTOM_DVE_SPECS[name] = spec
    return op


EXPA_P1 = _register(
    "EXPA_P1",
    Spec(body=sq(sq(sq(sq((Src0 * C0 + C1) * Src0 + One)))), reference=_ref_p1),
)
EXPA_P2 = _register(
    "EXPA_P2",
    Spec(body=sq(sq(sq(sq(Src0)))), accum=add, accum_init=Zero, reference=_ref_p2),
)


# ---- kernel build --------------------------------------------------------- #

# Per-block PSUM plan over 12500 class columns.  Each 512-col slot is a
# matmul; slots pack into PSUM tiles of PS_W columns which are consumed
# whole by either ACT (exp+accum, 'A') or the DVE custom exp ('D').  The
# final tile holds the two 106-col remainders at bank-aligned offsets.
PS_W = 1024                      # psum tile width (multiple of 512)
PS_BUFS = 4                      # pool depth (PS_W/512 banks each)
KINDS = "AADAADAADAAD"           # consumer per full tile (len 12288*2/PS_W)
REM = CS2 % 512                  # leftover columns per half (106)


def _block_slots():
    """Slot plan per block: list of tiles, each a list of
    (tile_col_off, half, class_lo, width)."""
    nfull = (CS2 // 512) * 2     # full 512 slots across both halves
    per_tile = PS_W // 512
    tiles = []
    eh = [0, 0]
    half = 0
    for t in range(nfull // per_tile):
        slots = []
        for s in range(per_tile):
            # alternate halves; skip a half when exhausted
            if eh[half] + 512 > (CS2 // 512) * 512:
                half ^= 1
            slots.append((s * 512, half, eh[half], 512))
            eh[half] += 512
            half ^= 1
        tiles.append(slots)
    rem = [(0, 0, eh[0], REM), (512, 1, eh[1], REM)]
    return tiles, rem


def _build(nloop=1, unroll=False):
    nc = bacc.Bacc("TRN2", target_bir_lowering=False, debug=False)

    xt2_d = nc.dram_tensor("xt2", [2 * D, B], F32, kind="ExternalInput").ap()
    pth_d = nc.dram_tensor("pth", [2 * D, CS2], F32, kind="ExternalInput").ap()
    xsb_d = nc.dram_tensor("xsb", [BS, D], F32, kind="ExternalInput").ap()
    pp_d = nc.dram_tensor("pp", [BS, D], F32, kind="ExternalInput").ap()
    s_d = nc.dram_tensor("s_out", [NBLK, 128], F32, kind="ExternalOutput").ap()
    pos_d = nc.dram_tensor("pos_out", [BS], F32, kind="ExternalOutput").ap()

    with tile.TileContext(nc) as tc:
        with (
            tc.tile_pool(name="res", bufs=1) as res,
            tc.tile_pool(name="sq", bufs=2) as sqp,
            tc.tile_pool(name="lnp", bufs=2) as lnp,
            tc.tile_pool(name="rsp", bufs=2) as rsp,
            tc.tile_pool(name="sml", bufs=2) as sml,
            tc.tile_pool(name="scr", bufs=2) as scr,
            tc.tile_pool(name="ptnp", bufs=2) as ptnp,
            tc.tile_pool(name="wsp", bufs=2) as wsp,
            tc.tile_pool(name="ps", bufs=PS_BUFS, space="PSUM") as psp,
        ):
            xsb = res.tile([BS, D], F32, tag="xsb")
            pp = res.tile([BS, D], F32, tag="pp")
            nc.sync.dma_start(xsb[:], xsb_d[:])
            nc.sync.dma_start(pp[:], pp_d[:])
            xt2 = res.tile([2 * D, B], F32, tag="xt2")
            nc.sync.dma_start(xt2[:], xt2_d[:])
            xt2r = res.tile([2 * D, B], F32R, tag="xt2r")
            nc.vector.tensor_copy(xt2r[:], xt2[:])
            # block-diagonal ones weights for per-half norms reduction
            obdf = res.tile([128, 128], F32, tag="obdf")
            nc.vector.memset(obdf[:], 0.0)
            nc.vector.memset(obdf[0:D, 0:D], 1.0)
            nc.vector.memset(obdf[D:128, D:128], 1.0)
            obd = res.tile([128, 128], F32R, tag="obd")
            nc.vector.tensor_copy(obd[:], obdf[:])
            bias_t = res.tile([128, 1], F32, tag="bias")
            nc.vector.memset(bias_t[:], float(np.log(2.0)))
            # proxies: both halves stacked, classes split (no duplication)
            pth = res.tile([2 * D, CS2], F32, tag="pth")
            # chunked input DMA so normalize can start early
            for o in range(0, CS2, 2048):
                w = min(2048, CS2 - o)
                nc.sync.dma_start(pth[:, o:o + w], pth_d[:, o:o + w])

            def body():
                # ---- positive term: pos = 2 (x.p_t)/||p_t||, [128, 1] ----
                xp = sml.tile([BS, D], F32, tag="xp")
                nc.vector.tensor_tensor(xp[:], xsb[:], pp[:], op=MULT)
                dot = sml.tile([BS, 1], F32, tag="dot")
                nc.vector.reduce_sum(dot[:], xp[:], axis=AX)
                pp2 = sml.tile([BS, D], F32, tag="xp")
                nc.vector.tensor_tensor(pp2[:], pp[:], pp[:], op=MULT)
                pn2 = sml.tile([BS, 1], F32, tag="pn2")
                nc.vector.reduce_sum(pn2[:], pp2[:], axis=AX)
                lnp2 = sml.tile([BS, 1], F32, tag="lnp2")
                nc.scalar.activation(lnp2[:], pn2[:], LN)
                rp = sml.tile([BS, 1], F32, tag="rp")
                nc.scalar.activation(rp[:], lnp2[:], EXP, scale=-0.5,
                                     bias=bias_t[0:BS])
                pos = sml.tile([BS, 1], F32, tag="pos")
                nc.vector.tensor_tensor(pos[:], dot[:], rp[:], op=MULT)
                nc.sync.dma_start(pos_d[:], pos[:, 0])

                # ---- normalize: n2 per half via block-diag ones matmul,
                # rs = exp(-0.5 ln n2 + ln 2) = 2/||p||, ptn = pth * rs.
                # ptn is double-buffered so the next iteration's normalize
                # can overlap this iteration's tail blocks ----
                ptn = ptnp.tile([2 * D, CS2], F32R, tag="ptn")
                for o in range(0, CS2, PS_W):
                    w = min(PS_W, CS2 - o)
                    sqt = sqp.tile([128, PS_W], F32R, tag="sq")
                    nc.vector.tensor_tensor(sqt[:, 0:w], pth[:, o:o + w],
                                            pth[:, o:o + w], op=MULT)
                    psn = psp.tile([128, PS_W], F32, tag="ps")
                    for c0 in range(0, w, 512):
                        cw = min(512, w - c0)
                        nc.tensor.matmul(psn[:, c0:c0 + cw], obd[:],
                                         sqt[:, c0:c0 + cw],
                                         start=True, stop=True)
                    lnt = lnp.tile([128, PS_W], F32, tag="ln")
                    nc.scalar.activation(lnt[:, 0:w], psn[:, 0:w], LN)
                    rs = rsp.tile([128, PS_W], F32, tag="rs")
                    nc.scalar.activation(rs[:, 0:w], lnt[:, 0:w], EXP,
                                         scale=-0.5, bias=bias_t[:])
                    nc.vector.tensor_tensor(ptn[:, o:o + w], pth[:, o:o + w],
                                            rs[:, 0:w], op=MULT)

                # ---- main: G = 2 X.P_hat per 128-row block; exp+sum
                # split between ACT ('A') and the DVE custom exp ('D') ----
                tiles_plan, rem_plan = _block_slots()
                for m in range(NBLK):
                    sums = sml.tile([128, 16], F32, tag="sums")
                    xh = (xt2r[0:D, 128 * m:128 * (m + 1)],
                          xt2r[D:128, 128 * m:128 * (m + 1)])

                    def fill(ps, slots):
                        for off, half, lo, w in slots:
                            nc.tensor.matmul(
                                ps[:, off:off + w], xh[half],
                                ptn[64 * half:64 * half + D, lo:lo + w],
                                start=True, stop=True,
                                tile_position=(64 * half, 0))

                    na = 0
                    dc = 0
                    ws = None
                    for ti, slots in enumerate(tiles_plan):
                        ps = psp.tile([128, PS_W], F32, tag="ps")
                        fill(ps, slots)
                        if KINDS[ti] == "A":
                            # exp in place (ScE->PSUM is the fast port)
                            nc.scalar.activation(ps[:, 0:PS_W], ps[:, 0:PS_W],
                                                 EXP,
                                                 accum_out=sums[:, na:na + 1])
                            na += 1
                        else:
                            if dc % 2 == 0:
                                ws = wsp.tile([128, 2 * PS_W], F32, tag="ws")
                            half_off = (dc % 2) * PS_W
                            nc.vector._custom_dve(
                                EXPA_P1,
                                out=ws[:, half_off:half_off + PS_W],
                                in0=ps[:, 0:PS_W], s0=C0V, s1=C1V)
                            if dc % 2 == 1:
                                nc.vector._custom_dve(
                                    EXPA_P2, out=ws[:], in0=ws[:],
                                    accum_out=sums[:, na:na + 1])
                                na += 1
                            dc += 1
                    if dc % 2 == 1:
                        nc.vector._custom_dve(
                            EXPA_P2, out=ws[:, 0:PS_W], in0=ws[:, 0:PS_W],
                            accum_out=sums[:, na:na + 1])
                        na += 1
                    # remainder tile: two 106-col slots at bank-aligned 0/512
                    ps = psp.tile([128, PS_W], F32, tag="ps")
                    fill(ps, rem_plan)
                    for off, half, lo, w in rem_plan:
                        nc.scalar.activation(ps[:, off:off + w],
                                             ps[:, off:off + w], EXP,
                                             accum_out=sums[:, na:na + 1])
                        na += 1
                    sblk = sml.tile([128, 1], F32, tag="sblk")
                    nc.vector.reduce_sum(sblk[:], sums[:, 0:na], axis=AX)
                    nc.sync.dma_start(s_d[m], sblk[:, 0])

            if unroll:
                for _ in range(nloop):
                    body()
            elif nloop == 1:
                body()
            else:
                # For_i carries an all-engine barrier per iteration; put U
                # bodies in the loop so the drain/refill amortizes 1/U.
                U = 10 if nloop % 10 == 0 else 1
                with tc.For_i(0, nloop // U, 1):
                    for _ in range(U):
                        body()

    nc.compile()
    return nc


def _get_nc(nloop=1):
    if nloop not in _CACHE:
        _CACHE[nloop] = _build(nloop)
    return _CACHE[nloop]


def _in_maps(xs, ts, proxies):
    xs = np.ascontiguousarray(xs, dtype=np.float32)
    proxies = np.ascontiguousarray(proxies, dtype=np.float32)
    ts = np.asarray(ts).astype(np.int64)
    xt = np.ascontiguousarray(xs.T)                  # [64, 1024]
    xt2 = np.concatenate([xt, xt], axis=0)           # [128, 1024]
    pt_all = np.ascontiguousarray(proxies.T)         # [64, 100000]
    ppos = proxies[ts]                               # [1024, 64]
    maps = []
    for c in range(NCORES):
        lo = c * CS
        pth = np.concatenate(
            [pt_all[:, lo:lo + CS2], pt_all[:, lo + CS2:lo + CS]], axis=0)
        maps.append({
            "xt2": xt2,
            "pth": np.ascontiguousarray(pth),
            "xsb": xs[c * BS:(c + 1) * BS],
            "pp": np.ascontiguousarray(ppos[c * BS:(c + 1) * BS]),
        })
    return maps


def _combine(results, ts=None):
    s = np.zeros(B, dtype=np.float64)
    pos = np.zeros(B, dtype=np.float64)
    for c in range(NCORES):
        s += results[c]["s_out"].reshape(B).astype(np.float64)
        pos[c * BS:(c + 1) * BS] = results[c]["pos_out"].astype(np.float64)
    r = s - np.exp(pos)
    loss = np.mean(-pos + np.log(r))
    return np.asarray(loss, dtype=np.float32)


def kernel(xs, ts, proxies):
    nc = _get_nc()
    maps = _in_maps(xs, ts, proxies)
    results = run_bass_kernel_spmd(nc, maps, list(range(NCORES))).results
    return _combine(results, ts)


if __name__ == "__main__":
    rng = np.random.default_rng(0)
    xs = rng.standard_normal((B, D)).astype(np.float32)
    ts = rng.integers(0, C, B)
    proxies = rng.standard_normal((C, D)).astype(np.float32)
    print(kernel(xs=xs, ts=ts, proxies=proxies))


# revision 21
# speedup vs baseline: 1.6915x; 1.6915x over previous
"""ProxyNCA loss on 8 Trainium2 NeuronCores.

Math: with p_hat = p / ||p||, the reference
    loss_i = D2[i,t_i] + log sum_{k != t_i} exp(-D2[i,k])
with D2 = |x|^2 + |p_hat|^2 - 2 x.p_hat collapses (|x|^2 and |p_hat|^2 = 1
cancel between the two terms) to
    loss_i = -G[i,t_i] + log sum_{k != t_i} exp(G[i,k]),   G = 2 X Pn^T.

Device sharding: proxies split over classes across 8 cores (12500/core),
further split across SBUF partition halves (6250 columns on partitions
0-63 holding classes [0,6250), 6250 on partitions 64-127 holding classes
[6250,12500)) so nothing is duplicated. Per core:
  - norms^2 via a block-diagonal ones matmul (one 128-contraction matmul
    per 512 columns gives per-half norms in each partition half),
  - 1/||p|| via ACT Ln+Exp (exp(-0.5 ln n2 + ln 2) = 2/||p||) -- stays in
    the natural_log_exp table set with the main Exp, so no ACT table
    reloads inside the loop,
  - G computed as row-packed fp32r matmul pairs (tile_position (0,0) and
    (64,0)) into [128,2048] PSUM groups,
  - sum_k exp(G) split between two engines: ACT does groups 0-3 + the
    212-col remainder with fused exp+accumulate; the DVE does groups 4-5
    with a custom 2-pass op: pass1 w = (1 + u(c1 + u c2))^16 ~= e^(G/16)
    (deg-2 poly in u = G/256 + 4 squarings, 8 ALU stages), pass2
    w^16 + fused row-sum (4 squarings + accum stage).
The positive term G[i, t_i] is computed per batch shard from host-gathered
proxy rows. Host combines in float64: subtracts exp(pos) from the global
sum (exact masking) and averages.
"""

import numpy as np
from operator import add

import concourse.bacc as bacc
import concourse.mybir as mybir
import concourse.tile as tile
from concourse.bass_utils import run_bass_kernel_spmd

import concourse.dve_ops as dve_ops
import concourse.bacc as _bacc_mod
from concourse.hw_specs import get_activation_tables as _get_act_tables


def _act_tables_lnexp_first(arch):
    """Steer the ACT table-load pass to the combined natural_log_exp set so
    Ln+Exp kernels load one table instead of thrashing between natural_log
    and exp_and_others.  Order (and thus act_func_set_id numbering) is
    preserved -- ln/exp are only hidden from the non-combined sets so the
    chooser can't pick them."""
    import concourse.mybir as _mb
    t = _get_act_tables(arch)
    ln_f = _mb.ActivationFunctionType.Ln
    exp_f = _mb.ActivationFunctionType.Exp
    out = {}
    for k, fns in t.items():
        if "natural_log_exp" in k:
            out[k] = fns
        else:
            out[k] = {f for f in fns if f not in (ln_f, exp_f)}
    return out


_bacc_mod.get_activation_tables = _act_tables_lnexp_first
from concourse.dve_spec import Spec, Src0, C0, C1, Zero, One, lower, _has_src1, sq
from concourse.dve_uop import DveOpSpec

F32 = mybir.dt.float32
F32R = mybir.dt.float32r
AX = mybir.AxisListType.X
MULT = mybir.AluOpType.mult
EXP = mybir.ActivationFunctionType.Exp
LN = mybir.ActivationFunctionType.Ln

B, C, D = 1024, 100000, 64
NCORES = 8
CS = C // NCORES          # 12500 classes per core
CS2 = CS // 2             # 6250 classes per partition half
BS = B // NCORES          # 128 batch rows per core (positive extraction)
NBLK = B // 128           # 8 batch blocks of 128 rows

# exp(g) ~= ((1 + u(C1 + u C0))^16)^16 with u = g/256 (Taylor in u).
SCALE = 256.0
C1V = 1.0 / SCALE
C0V = 1.0 / (2.0 * SCALE * SCALE)

_CACHE = {}


# ---- custom DVE ops ------------------------------------------------------- #

def _ref_p1(in0, in1, c0, c1, c2):
    x = in0.astype(np.float32)
    c0 = np.float32(c0) if not isinstance(c0, np.ndarray) else c0.astype(np.float32)
    c1 = np.float32(c1) if not isinstance(c1, np.ndarray) else c1.astype(np.float32)
    w = ((x * c0 + c1) * x + np.float32(1.0)).astype(np.float32)
    for _ in range(4):
        w = (w * w).astype(np.float32)
    return w


def _ref_p2(in0, in1, c0, c1, c2):
    w = in0.astype(np.float32)
    for _ in range(4):
        w = (w * w).astype(np.float32)
    s = w.reshape(w.shape[0], -1).sum(axis=-1, keepdims=True)
    return w, s


def _register(name, spec, subdim=False):
    if name in dve_ops._SUB_OPCODE_FOR_NAME:
        for op in dve_ops.OPS:
            if op.name == name:
                return op
        raise RuntimeError(f"{name} registered but not in OPS")
    row = dve_ops._CUSTOM_DVE_ROW_BASE + len(dve_ops.OPS)
    assert row < 0x20
    dve_ops._SUB_OPCODE_FOR_NAME[name] = row
    shas = {}
    for ver in ("v3", "v4"):
        uops = lower(spec, ver=ver)
        shas[ver] = DveOpSpec(
            name=name, opcode=row, uops=uops, rd1_en=_has_src1(spec)
        ).sha(ver)
    op = dve_ops.DveOp(name, spec, subdim=subdim, uops_sha=shas)
    dve_ops.OPS.append(op)
    dve_ops.CUSTOM_DVE_SPECS[name] = spec
    return op


EXPA_P1 = _register(
    "EXPA_P1",
    Spec(body=sq(sq(sq(sq((Src0 * C0 + C1) * Src0 + One)))), reference=_ref_p1),
)
EXPA_P2 = _register(
    "EXPA_P2",
    Spec(body=sq(sq(sq(sq(Src0)))), accum=add, accum_init=Zero, reference=_ref_p2),
)


# ---- kernel build --------------------------------------------------------- #

# Per-block PSUM plan over 12500 class columns.  Each 512-col slot is a
# matmul; slots pack into PSUM tiles of PS_W columns which are consumed
# whole by either ACT (exp+accum, 'A') or the DVE custom exp ('D').  The
# final tile holds the two 106-col remainders at bank-aligned offsets.
PS_W = 1024                      # psum tile width (multiple of 512)
PS_BUFS = 4                      # pool depth (PS_W/512 banks each)
KINDS = "AADAADAADAAD"           # consumer per full tile (len 12288*2/PS_W)
REM = CS2 % 512                  # leftover columns per half (106)


def _block_slots():
    """Slot plan per block: list of tiles, each a list of
    (tile_col_off, half, class_lo, width)."""
    nfull = (CS2 // 512) * 2     # full 512 slots across both halves
    per_tile = PS_W // 512
    tiles = []
    eh = [0, 0]
    half = 0
    for t in range(nfull // per_tile):
        slots = []
        for s in range(per_tile):
            # alternate halves; skip a half when exhausted
            if eh[half] + 512 > (CS2 // 512) * 512:
                half ^= 1
            slots.append((s * 512, half, eh[half], 512))
            eh[half] += 512
            half ^= 1
        tiles.append(slots)
    rem = [(0, 0, eh[0], REM), (512, 1, eh[1], REM)]
    return tiles, rem


def _build(nloop=1, unroll=False):
    nc = bacc.Bacc("TRN2", target_bir_lowering=False, debug=False)

    xt2_d = nc.dram_tensor("xt2", [2 * D, B], F32, kind="ExternalInput").ap()
    pth_d = nc.dram_tensor("pth", [2 * D, CS2], F32, kind="ExternalInput").ap()
    xsb_d = nc.dram_tensor("xsb", [BS, D], F32, kind="ExternalInput").ap()
    pp_d = nc.dram_tensor("pp", [BS, D], F32, kind="ExternalInput").ap()
    s_d = nc.dram_tensor("s_out", [NBLK, 128], F32, kind="ExternalOutput").ap()
    pos_d = nc.dram_tensor("pos_out", [BS], F32, kind="ExternalOutput").ap()

    with tile.TileContext(nc) as tc:
        with (
            tc.tile_pool(name="res", bufs=1) as res,
            tc.tile_pool(name="sq", bufs=2) as sqp,
            tc.tile_pool(name="lnp", bufs=2) as lnp,
            tc.tile_pool(name="rsp", bufs=2) as rsp,
            tc.tile_pool(name="sml", bufs=2) as sml,
            tc.tile_pool(name="scr", bufs=2) as scr,
            tc.tile_pool(name="ptnp", bufs=2) as ptnp,
            tc.tile_pool(name="wsp", bufs=2) as wsp,
            tc.tile_pool(name="ps", bufs=PS_BUFS, space="PSUM") as psp,
        ):
            xsb = res.tile([BS, D], F32, tag="xsb")
            pp = res.tile([BS, D], F32, tag="pp")
            nc.sync.dma_start(xsb[:], xsb_d[:])
            nc.sync.dma_start(pp[:], pp_d[:])
            xt2 = res.tile([2 * D, B], F32, tag="xt2")
            nc.sync.dma_start(xt2[:], xt2_d[:])
            xt2r = res.tile([2 * D, B], F32R, tag="xt2r")
            nc.vector.tensor_copy(xt2r[:], xt2[:])
            # block-diagonal ones weights for per-half norms reduction
            obdf = res.tile([128, 128], F32, tag="obdf")
            nc.vector.memset(obdf[:], 0.0)
            nc.vector.memset(obdf[0:D, 0:D], 1.0)
            nc.vector.memset(obdf[D:128, D:128], 1.0)
            obd = res.tile([128, 128], F32R, tag="obd")
            nc.vector.tensor_copy(obd[:], obdf[:])
            bias_t = res.tile([128, 1], F32, tag="bias")
            nc.vector.memset(bias_t[:], float(np.log(2.0)))
            # proxies: both halves stacked, classes split (no duplication)
            pth = res.tile([2 * D, CS2], F32, tag="pth")
            # chunked input DMA so normalize can start early
            for o in range(0, CS2, 2048):
                w = min(2048, CS2 - o)
                nc.sync.dma_start(pth[:, o:o + w], pth_d[:, o:o + w])

            def body():
                # ---- positive term: pos = 2 (x.p_t)/||p_t||, [128, 1] ----
                xp = sml.tile([BS, D], F32, tag="xp")
                nc.vector.tensor_tensor(xp[:], xsb[:], pp[:], op=MULT)
                dot = sml.tile([BS, 1], F32, tag="dot")
                nc.vector.reduce_sum(dot[:], xp[:], axis=AX)
                pp2 = sml.tile([BS, D], F32, tag="xp")
                nc.vector.tensor_tensor(pp2[:], pp[:], pp[:], op=MULT)
                pn2 = sml.tile([BS, 1], F32, tag="pn2")
                nc.vector.reduce_sum(pn2[:], pp2[:], axis=AX)
                lnp2 = sml.tile([BS, 1], F32, tag="lnp2")
                nc.scalar.activation(lnp2[:], pn2[:], LN)
                rp = sml.tile([BS, 1], F32, tag="rp")
                nc.scalar.activation(rp[:], lnp2[:], EXP, scale=-0.5,
                                     bias=bias_t[0:BS])
                pos = sml.tile([BS, 1], F32, tag="pos")
                nc.vector.tensor_tensor(pos[:], dot[:], rp[:], op=MULT)
                nc.sync.dma_start(pos_d[:], pos[:, 0])

                # ---- normalize: n2 per half via block-diag ones matmul,
                # rs = exp(-0.5 ln n2 + ln 2) = 2/||p||, ptn = pth * rs.
                # ptn is double-buffered so the next iteration's normalize
                # can overlap this iteration's tail blocks ----
                ptn = ptnp.tile([2 * D, CS2], F32R, tag="ptn")
                for o in range(0, CS2, PS_W):
                    w = min(PS_W, CS2 - o)
                    sqt = sqp.tile([128, PS_W], F32R, tag="sq")
                    nc.vector.tensor_tensor(sqt[:, 0:w], pth[:, o:o + w],
                                            pth[:, o:o + w], op=MULT)
                    psn = psp.tile([128, PS_W], F32, tag="ps")
                    for c0 in range(0, w, 512):
                        cw = min(512, w - c0)
                        nc.tensor.matmul(psn[:, c0:c0 + cw], obd[:],
                                         sqt[:, c0:c0 + cw],
                                         start=True, stop=True)
                    lnt = lnp.tile([128, PS_W], F32, tag="ln")
                    nc.scalar.activation(lnt[:, 0:w], psn[:, 0:w], LN)
                    rs = rsp.tile([128, PS_W], F32, tag="rs")
                    nc.scalar.activation(rs[:, 0:w], lnt[:, 0:w], EXP,
                                         scale=-0.5, bias=bias_t[:])
                    nc.vector.tensor_tensor(ptn[:, o:o + w], pth[:, o:o + w],
                                            rs[:, 0:w], op=MULT)

                # ---- main: G = 2 X.P_hat per 128-row block; exp+sum
                # split between ACT ('A') and the DVE custom exp ('D') ----
                tiles_plan, rem_plan = _block_slots()
                for m in range(NBLK):
                    sums = sml.tile([128, 16], F32, tag="sums")
                    xh = (xt2r[0:D, 128 * m:128 * (m + 1)],
                          xt2r[D:128, 128 * m:128 * (m + 1)])

                    def fill(ps, slots):
                        for off, half, lo, w in slots:
                            nc.tensor.matmul(
                                ps[:, off:off + w], xh[half],
                                ptn[64 * half:64 * half + D, lo:lo + w],
                                start=True, stop=True,
                                tile_position=(64 * half, 0))

                    na = 0
                    dc = 0
                    ws = None
                    for ti, slots in enumerate(tiles_plan):
                        ps = psp.tile([128, PS_W], F32, tag="ps")
                        fill(ps, slots)
                        if KINDS[ti] == "A":
                            sc = scr.tile([128, PS_W], F32, tag="sc")
                            nc.scalar.activation(sc[:, 0:PS_W], ps[:, 0:PS_W],
                                                 EXP,
                                                 accum_out=sums[:, na:na + 1])
                            na += 1
                        else:
                            if dc % 2 == 0:
                                ws = wsp.tile([128, 2 * PS_W], F32, tag="ws")
                            half_off = (dc % 2) * PS_W
                            nc.vector._custom_dve(
                                EXPA_P1,
                                out=ws[:, half_off:half_off + PS_W],
                                in0=ps[:, 0:PS_W], s0=C0V, s1=C1V)
                            if dc % 2 == 1:
                                nc.vector._custom_dve(
                                    EXPA_P2, out=ws[:], in0=ws[:],
                                    accum_out=sums[:, na:na + 1])
                                na += 1
                            dc += 1
                    if dc % 2 == 1:
                        nc.vector._custom_dve(
                            EXPA_P2, out=ws[:, 0:PS_W], in0=ws[:, 0:PS_W],
                            accum_out=sums[:, na:na + 1])
                        na += 1
                    # remainder tile: two 106-col slots at bank-aligned 0/512
                    ps = psp.tile([128, PS_W], F32, tag="ps")
                    fill(ps, rem_plan)
                    scm = scr.tile([128, PS_W], F32, tag="sc")
                    for off, half, lo, w in rem_plan:
                        nc.scalar.activation(scm[:, off:off + w],
                                             ps[:, off:off + w], EXP,
                                             accum_out=sums[:, na:na + 1])
                        na += 1
                    sblk = sml.tile([128, 1], F32, tag="sblk")
                    nc.vector.reduce_sum(sblk[:], sums[:, 0:na], axis=AX)
                    nc.sync.dma_start(s_d[m], sblk[:, 0])

            if unroll:
                for _ in range(nloop):
                    body()
            elif nloop == 1:
                body()
            else:
                # For_i carries an all-engine barrier per iteration; put U
                # bodies in the loop so the drain/refill amortizes 1/U.
                U = 10 if nloop % 10 == 0 else 1
                with tc.For_i(0, nloop // U, 1):
                    for _ in range(U):
                        body()

    nc.compile()
    return nc


def _get_nc(nloop=1):
    if nloop not in _CACHE:
        _CACHE[nloop] = _build(nloop)
    return _CACHE[nloop]


def _in_maps(xs, ts, proxies):
    xs = np.ascontiguousarray(xs, dtype=np.float32)
    proxies = np.ascontiguousarray(proxies, dtype=np.float32)
    ts = np.asarray(ts).astype(np.int64)
    xt = np.ascontiguousarray(xs.T)                  # [64, 1024]
    xt2 = np.concatenate([xt, xt], axis=0)           # [128, 1024]
    pt_all = np.ascontiguousarray(proxies.T)         # [64, 100000]
    ppos = proxies[ts]                               # [1024, 64]
    maps = []
    for c in range(NCORES):
        lo = c * CS
        pth = np.concatenate(
            [pt_all[:, lo:lo + CS2], pt_all[:, lo + CS2:lo + CS]], axis=0)
        maps.append({
            "xt2": xt2,
            "pth": np.ascontiguousarray(pth),
            "xsb": xs[c * BS:(c + 1) * BS],
            "pp": np.ascontiguousarray(ppos[c * BS:(c + 1) * BS]),
        })
    return maps


def _combine(results, ts=None):
    s = np.zeros(B, dtype=np.float64)
    pos = np.zeros(B, dtype=np.float64)
    for c in range(NCORES):
        s += results[c]["s_out"].reshape(B).astype(np.float64)
        pos[c * BS:(c + 1) * BS] = results[c]["pos_out"].astype(np.float64)
    r = s - np.exp(pos)
    loss = np.mean(-pos + np.log(r))
    return np.asarray(loss, dtype=np.float32)


def kernel(xs, ts, proxies):
    nc = _get_nc()
    maps = _in_maps(xs, ts, proxies)
    results = run_bass_kernel_spmd(nc, maps, list(range(NCORES))).results
    return _combine(results, ts)


if __name__ == "__main__":
    rng = np.random.default_rng(0)
    xs = rng.standard_normal((B, D)).astype(np.float32)
    ts = rng.integers(0, C, B)
    proxies = rng.standard_normal((C, D)).astype(np.float32)
    print(kernel(xs=xs, ts=ts, proxies=proxies))
